# revision 2
# baseline (speedup 1.0000x reference)
"""Fused ASPPGraphFusion kernel for 8 Trainium2 NeuronCores.

Math: with A_hat = ones(5,5)/5, fused_nodes[b,i,c] is identical for all i:
    m[b,c] = mean_j(node_feats[b,j] @ gcn_w)[c] + gcn_b[c]
so  out = sum_i sm_i * f_i * m  = m * (sm1*f1 + ... + sm5*f5)
and the final 1x1 conv folds into per-sample weights:
    final[co] = sum_c (fusion_w[co,c]*m[c]) * S[c] + chat[co]
where S = merged 25-tap conv of x (no bias), taps = union of the four
conv branches scaled by softmax weights, and chat absorbs all biases and
the (constant-per-sample) global-average branch f5.

node_feats (per-branch spatial means) only need rectangle sums of x:
    R(oh,ow) = T - excluded row sums - excluded col sums + corner pixels
so launch 1 computes per channel: total T, the 6 edge row sums, the 6
edge col sums (rows/cols 0..2 and 381..383); raw 6x6 corner pixels are
DMA'd directly.  Launch 2 runs the merged conv + folded 1x1.
"""

import numpy as np
from contextlib import ExitStack

import concourse.bass as bass
import concourse.bacc as bacc
import concourse.tile as tile
from concourse import mybir
from concourse.bass_utils import run_bass_kernel_spmd

F32 = mybir.dt.float32
B, CIN, CMID, COUT, H, W = 8, 32, 16, 32, 384, 384
NPIX = H * W
NCORES = 8
DIL = {1: 1, 2: 2, 3: 3}  # branch index (w2,w3,w4) -> dilation

# 25 distinct tap offsets {0,+-1}^2 u {0,+-2}^2 u {0,+-3}^2
TAPS = sorted({(d * (kh - 1), d * (kw - 1))
               for d in (1, 2, 3) for kh in range(3) for kw in range(3)})
NTAP = len(TAPS)  # 25
assert NTAP == 25

# ---- launch 2 geometry ----
RT = 16                 # output rows per row-tile
NTILE = H // RT         # 24 row-tiles
XROWS = RT + 6          # 22 rows incl. 3-halo each side
XCOLS = 404             # 7 zero | 384 data | 13 zero
DCOL = 7                # first data col in xpad
SCOLS = 396             # stage width: padded output row (data at 3..386)


def _np(x):
    return np.asarray(x)


def _build_reduce_nc():
    nc = bacc.Bacc("TRN2", target_bir_lowering=False, debug=False,
                   num_devices=NCORES)
    x = nc.dram_tensor("x", [CIN, H, W], F32, kind="ExternalInput").ap()
    emat = nc.dram_tensor("emat", [128, 24], F32, kind="ExternalInput").ap()
    o_red = nc.dram_tensor("o_red", [8, CIN], F32, kind="ExternalOutput").ap()
    o_cs = nc.dram_tensor("o_cs", [1, CIN * 6], F32, kind="ExternalOutput").ap()
    o_corn = nc.dram_tensor("o_corn", [CIN, 36], F32, kind="ExternalOutput").ap()

    with tile.TileContext(nc) as tc:
        with ExitStack() as ctx:
            cpool = ctx.enter_context(tc.tile_pool(name="chunks", bufs=3))
            ppool = ctx.enter_context(tc.tile_pool(name="ps", bufs=2, space="PSUM"))
            spool = ctx.enter_context(tc.tile_pool(name="stage", bufs=1))

            e_sb = spool.tile([128, 24], F32)
            nc.sync.dma_start(e_sb[:], emat[:])
            st_red = spool.tile([8, CIN], F32)
            st_cs = spool.tile([1, CIN * 6], F32)

            for cin in range(CIN):
                ps = ppool.tile([8, W], F32)
                for k in range(3):
                    ch = cpool.tile([128, W], F32)
                    nc.sync.dma_start(ch[:], x[cin, 128 * k:128 * (k + 1), :])
                    nc.tensor.matmul(ps[:, :], e_sb[:, 8 * k:8 * k + 8],
                                     ch[:, :], start=(k == 0), stop=(k == 2))
                # rows of ps: 0 = col-sums over h (full), 1..3 = raw rows 0..2,
                # 4..6 = raw rows 381..383
                nc.vector.tensor_reduce(st_red[0:7, cin:cin + 1], ps[0:7, :],
                                        axis=mybir.AxisListType.X,
                                        op=mybir.AluOpType.add)
                nc.vector.tensor_copy(st_cs[0:1, cin * 6:cin * 6 + 3], ps[0:1, 0:3])
                nc.vector.tensor_copy(st_cs[0:1, cin * 6 + 3:cin * 6 + 6],
                                      ps[0:1, W - 3:W])

            nc.sync.dma_start(o_red[0:8, :], st_red[:])
            nc.sync.dma_start(o_cs[:], st_cs[:])
            for q, (r0, c0) in enumerate([(0, 0), (0, W - 3), (H - 3, 0),
                                          (H - 3, W - 3)]):
                nc.sync.dma_start(o_corn[:, 9 * q:9 * q + 9],
                                  x[:, r0:r0 + 3, c0:c0 + 3])
    nc.compile()
    return nc


def _build_conv_nc():
    nc = bacc.Bacc("TRN2", target_bir_lowering=False, debug=False,
                   num_devices=NCORES)
    x = nc.dram_tensor("x", [CIN, H, W], F32, kind="ExternalInput").ap()
    tapw = nc.dram_tensor("tapw", [CIN, NTAP * 32], F32,
                          kind="ExternalInput").ap()
    fusw = nc.dram_tensor("fusw", [128, COUT], F32, kind="ExternalInput").ap()
    cvec = nc.dram_tensor("cvec", [COUT, 1], F32, kind="ExternalInput").ap()
    out = nc.dram_tensor("out", [COUT, H, W], F32, kind="ExternalOutput").ap()

    # quadrant assignment: PE column-group j handles taps j, j+4, ...
    quads = [[t for t in range(NTAP) if t % 4 == j] for j in range(4)]
    nround = max(len(q) for q in quads)

    with tile.TileContext(nc) as tc:
        with ExitStack() as ctx:
            wpool = ctx.enter_context(tc.tile_pool(name="w", bufs=1))
            xpool = ctx.enter_context(tc.tile_pool(name="xp", bufs=1))
            sgpool = ctx.enter_context(tc.tile_pool(name="sg", bufs=1))
            cppool = ctx.enter_context(tc.tile_pool(name="cp", bufs=3))
            pa = ctx.enter_context(tc.tile_pool(name="pa", bufs=2, space="PSUM"))
            pb = ctx.enter_context(tc.tile_pool(name="pb", bufs=2, space="PSUM"))

            tapw_sb = wpool.tile([CIN, NTAP * 32], F32)
            nc.sync.dma_start(tapw_sb[:], tapw[:])
            fusw_sb = wpool.tile([128, COUT], F32)
            nc.sync.dma_start(fusw_sb[:], fusw[:])
            cvec_sb = wpool.tile([COUT, 1], F32)
            nc.sync.dma_start(cvec_sb[:], cvec[:])

            # two persistent x buffers (manual double buffering) + stages
            xpads = [xpool.tile([CIN, XROWS, XCOLS], F32, tag=f"xp{i}",
                                name=f"xpad{i}") for i in range(2)]
            stages = [sgpool.tile([CIN, RT, SCOLS], F32, tag=f"sg{i}",
                                  name=f"stage{i}") for i in range(2)]
            for t in xpads:
                nc.gpsimd.memset(t[:], 0.0)

            for it in range(NTILE):
                h0 = it * RT
                xp = xpads[it % 2]
                sg = stages[it % 2]
                g0, g1 = max(0, h0 - 3), min(H, h0 + RT + 3)
                r0 = g0 - h0 + 3          # local row of first loaded row
                r1 = r0 + (g1 - g0)
                if it > 1 and r0 > 0:
                    nc.vector.memset(xp[:, 0:r0, :], 0.0)
                if it > 1 and r1 < XROWS:
                    nc.vector.memset(xp[:, r1:XROWS, :], 0.0)
                nc.sync.dma_start(xp[:, r0:r1, DCOL:DCOL + W], x[:, g0:g1, :])

                for r in range(RT):
                    acc = pa.tile([128, SCOLS], F32)
                    for rd in range(nround):
                        for j in range(4):
                            if rd >= len(quads[j]):
                                continue
                            t = quads[j][rd]
                            oh, ow = TAPS[t]
                            nc.tensor.matmul(
                                acc[32 * j:32 * j + 32, :],
                                tapw_sb[:, 32 * t:32 * t + 32],
                                xp[:, r + 3 + oh, 4 + ow:4 + ow + SCOLS],
                                start=(rd == 0),
                                stop=(rd == len(quads[j]) - 1),
                                tile_position=(0, 32 * j))
                    cp = cppool.tile([128, SCOLS], F32)
                    nc.vector.tensor_copy(cp[:], acc[:])
                    fin = pb.tile([COUT, SCOLS], F32)
                    nc.tensor.matmul(fin[:, :], fusw_sb[:, :], cp[:, :],
                                     start=True, stop=True,
                                     tile_position=(0, 0))
                    nc.scalar.activation(sg[:, r, :], fin[:, :],
                                         mybir.ActivationFunctionType.Identity,
                                         bias=cvec_sb[:, 0:1])
                for r in range(RT):
                    nc.sync.dma_start(out[:, h0 + r, :], sg[:, r, 3:3 + W])
    nc.compile()
    return nc


def _softmax(v):
    e = np.exp(v - np.max(v))
    return e / e.sum()


def _merged_taps(w1, w2, w3, w4, sm):
    """W~[(oh,ow)][cin, c] in float64."""
    Wm = {t: np.zeros((CIN, CMID)) for t in TAPS}
    Wm[(0, 0)] += sm[0] * w1[:, :, 0, 0].T.astype(np.float64)
    for i, wb in ((1, w2), (2, w3), (3, w4)):
        d = DIL[i]
        for kh in range(3):
            for kw in range(3):
                Wm[(d * (kh - 1), d * (kw - 1))] += (
                    sm[i] * wb[:, :, kh, kw].T.astype(np.float64))
    return Wm


def host_fold(inputs, red, cs_band, corners):
    """Per-sample folded weights from launch-1 reductions (all float64).

    red: [B, 8, CIN]; cs_band: [B, CIN, 6]; corners: [B, CIN, 36]
    returns tapw [CIN, NTAP*32] f32 (shared), fusw [B,128,COUT] f32,
    cvec [B,COUT,1] f32
    """
    sm = _softmax(inputs["attn_weights"].astype(np.float64))
    w_list = [inputs[f"w{i}"].astype(np.float64) for i in range(1, 6)]
    b_list = [inputs[f"b{i}"].astype(np.float64) for i in range(1, 6)]
    gcn_w = inputs["gcn_w"].astype(np.float64)
    gcn_b = inputs["gcn_b"].astype(np.float64)
    fw = inputs["fusion_w"].astype(np.float64)[:, :, 0, 0]
    fb = inputs["fusion_b"].astype(np.float64)

    Wm = _merged_taps(w_list[0], w_list[1], w_list[2], w_list[3], sm)
    tapw = np.zeros((CIN, NTAP * 32), np.float32)
    for t, (oh, ow) in enumerate(TAPS):
        tapw[:, 32 * t:32 * t + CMID] = Wm[(oh, ow)].astype(np.float32)

    band_h = [0, 1, 2, H - 3, H - 2, H - 1]
    fusw = np.zeros((B, 128, COUT), np.float32)
    cvec = np.zeros((B, COUT, 1), np.float32)
    for b in range(B):
        T = red[b, 0].astype(np.float64)                  # [CIN]
        rs = {band_h[k]: red[b, 1 + k].astype(np.float64) for k in range(6)}
        cs = {band_h[k]: cs_band[b, :, k].astype(np.float64) for k in range(6)}
        corn = corners[b].astype(np.float64).reshape(CIN, 4, 3, 3)

        def cornpx(h, w):
            qi = (0 if h < 3 else 2) + (0 if w < 3 else 1)
            return corn[:, qi, h if h < 3 else h - (H - 3),
                        w if w < 3 else w - (W - 3)]

        def rect(oh, ow):
            hex_ = list(range(0, oh)) if oh > 0 else list(range(H + oh, H))
            wex_ = list(range(0, ow)) if ow > 0 else list(range(W + ow, W))
            r = T.copy()
            for h in hex_:
                r -= rs[h]
            for w in wex_:
                r -= cs[w]
            for h in hex_:
                for w in wex_:
                    r += cornpx(h, w)
            return r  # [CIN]

        # node_feats: per-branch spatial means
        nf = np.zeros((5, CMID))
        nf[0] = (w_list[0][:, :, 0, 0] @ rect(0, 0)) / NPIX + b_list[0]
        for i, wb in ((1, w_list[1]), (2, w_list[2]), (3, w_list[3])):
            d = DIL[i]
            acc = np.zeros(CMID)
            for kh in range(3):
                for kw in range(3):
                    acc += wb[:, :, kh, kw] @ rect(d * (kh - 1), d * (kw - 1))
            nf[i] = acc / NPIX + b_list[i]
        f5c = w_list[4][:, :, 0, 0] @ (T / NPIX) + b_list[4]
        nf[4] = f5c

        m = (nf @ gcn_w).mean(axis=0) + gcn_b                    # [CMID]
        F = fw * m[None, :]                                      # [COUT,CMID]
        btil = sum(sm[i] * b_list[i] for i in range(4))
        K5 = btil + sm[4] * f5c
        chat = F @ K5 + fb
        for j in range(4):
            fusw[b, 32 * j:32 * j + CMID, :] = F.T.astype(np.float32)
        cvec[b, :, 0] = chat.astype(np.float32)
    return tapw, fusw, cvec


def _emat():
    e = np.zeros((128, 24), np.float32)
    for k in range(3):
        e[:, 8 * k] = 1.0
    for j in range(3):
        e[j, 1 + j] = 1.0            # chunk 0 rows 0..2
        e[125 + j, 16 + 4 + j] = 1.0  # chunk 2 rows 381..383
    return e


_NC_CACHE = {}


def kernel(**inputs):
    inputs = {k: _np(v) for k, v in inputs.items()}
    x = inputs["x"].astype(np.float32)

    if "reduce" not in _NC_CACHE:
        _NC_CACHE["reduce"] = _build_reduce_nc()
    nc1 = _NC_CACHE["reduce"]
    emat = _emat()
    in_maps1 = [{"x": x[b], "emat": emat} for b in range(B)]
    res1 = run_bass_kernel_spmd(nc1, in_maps1, list(range(NCORES))).results

    red = np.stack([res1[b]["o_red"] for b in range(B)])          # [B,8,CIN]
    cs_band = np.stack([res1[b]["o_cs"].reshape(CIN, 6) for b in range(B)])
    corners = np.stack([res1[b]["o_corn"] for b in range(B)])     # [B,CIN,36]

    tapw, fusw, cvec = host_fold(inputs, red, cs_band, corners)

    if "conv" not in _NC_CACHE:
        _NC_CACHE["conv"] = _build_conv_nc()
    nc2 = _NC_CACHE["conv"]
    in_maps2 = [{"x": x[b], "tapw": tapw, "fusw": fusw[b], "cvec": cvec[b]}
                for b in range(B)]
    res2 = run_bass_kernel_spmd(nc2, in_maps2, list(range(NCORES))).results
    return np.stack([res2[b]["out"] for b in range(B)])
